# revision 17
# baseline (speedup 1.0000x reference)
"""Trainium2 Bass kernel for nn_NSMCell (GNN message passing).

Strategy
--------
The reference output is only [N]: a per-graph blend of two segment softmaxes
over per-node scalars.  Both scalars are of the form

    s_i = sum_d w_d * elu( M_g[d, :] @ x_i )

where for "node items" M_g = (sim[g] . W_node_props) * instr[g] and x = node
attr, and for "edge items" M_g = W_edge * instr[g] and x = edge attr.  The
per-graph matrices are built on the host (they are tiny); the device streams
all item columns through 4 matmuls + exp/min elu + a weighted partition
reduce.  Graphs (16 per core) are sharded across the 8 cores; every graph's
node and edge items are padded to fixed per-graph segment sizes so a single
NEFF serves all cores.  The edge-message scatter (index_add) collapses to a
host-side bincount of per-edge scalars, and the segment softmax + blend run
on the host over [N] values (negligible work).

Device layout per 512-item tile (d on partitions, 2 chunks of 128):
  y[d, e]   = A_seg[k, d]^T @ xT[k, e]      4 matmuls -> PSUM f32
  E         = exp(y)                        ScalarE, PSUM -> SBUF bf16
  R1        = max(y, 0) + 1                 VectorE tensor_scalar chain
  EL1       = min(E, R1) = elu(y) + 1       VectorE tensor_tensor
  s-rows   += (w (x) delta_c)^T @ EL1       2 matmuls into a PSUM s-bank
The s-bank accumulates one 512-wide row per tile (col-group trick), drained
once at the end; host subtracts sum(w) to undo the +1.
"""

import numpy as np
import ml_dtypes

BF16 = ml_dtypes.bfloat16
N_CORES = 8
D = 256
TILE = 512  # items per tile


# ----------------------------------------------------------------------------
# Bass kernel builder (one NEFF shared by all cores)
# ----------------------------------------------------------------------------

_BASS_CACHE = {}


def _get_elup1_op():
    """Register (once) a custom fused DVE op: out = min(in0, relu(in1) + s0).

    With in0 = exp(y) and in1 = y this computes elu(y) + 1 in a single
    VectorE pass, replacing a tensor_scalar + tensor_tensor pair."""
    from concourse import dve_ops
    from concourse.dve_spec import (Spec, Src0, Src1, C0, relu, minn, lower,
                                    _has_src1)
    from concourse.dve_uop import DveOpSpec

    for o in dve_ops.OPS:
        if o.name == "ELUP1_ANT":
            return o

    def ref(in0, in1, s0, s1, imm2):
        return np.minimum(
            in0.astype(np.float32),
            np.maximum(in1.astype(np.float32), 0.0) + s0,
        ).astype(np.float32)

    spec = Spec(body=minn(Src0, relu(Src1) + C0), reference=ref)
    row = dve_ops._CUSTOM_DVE_ROW_BASE + len(dve_ops.OPS)
    shas = {}
    for ver in ("v3", "v4"):
        uops = lower(spec, ver=ver)
        shas[ver] = DveOpSpec(name="ELUP1_ANT", opcode=row, uops=uops,
                              rd1_en=_has_src1(spec)).sha(ver)
    op = dve_ops.DveOp("ELUP1_ANT", spec, subdim=False, uops_sha=shas)
    dve_ops.OPS.append(op)
    dve_ops.CUSTOM_DVE_SPECS[op.name] = op.spec
    dve_ops._SUB_OPCODE_FOR_NAME[op.name] = row
    return op


def _build_bass(n_tiles, tiles_per_seg, m_pad):
    """Build the Tile/Bass program.

    n_tiles: number of 512-item tiles per core (== 16 graphs * tiles_per_seg)
    tiles_per_seg: tiles per graph segment-pair (1 node tile + edge tiles)
    """
    key = (n_tiles, tiles_per_seg, m_pad)
    if key in _BASS_CACHE:
        return _BASS_CACHE[key]

    import concourse.mybir as mybir
    import concourse.tile as tile
    from concourse import bacc

    dt = mybir.dt
    n_seg = 32  # 16 graphs x (node, edge)
    assert n_tiles == 16 * tiles_per_seg
    assert m_pad == n_tiles * TILE
    assert n_tiles <= 128

    elup1 = _get_elup1_op()
    nc = bacc.Bacc("TRN2", target_bir_lowering=False)
    items_d = nc.dram_tensor("items", [2 * 128, m_pad], dt.bfloat16,
                             kind="ExternalInput")
    mats_d = nc.dram_tensor("mats", [128, n_seg * 2 * 2 * 128], dt.bfloat16,
                            kind="ExternalInput")
    wtab_d = nc.dram_tensor("wtab", [128, 2 * 2 * 32 * 32], dt.bfloat16,
                            kind="ExternalInput")
    s_d = nc.dram_tensor("s_out", [128, TILE], dt.float32,
                         kind="ExternalOutput")

    with tile.TileContext(nc) as tc:
        with (
            tc.tile_pool(name="const", bufs=1) as const_pool,
            tc.tile_pool(name="items", bufs=6) as item_pool,
            tc.tile_pool(name="psum_y", bufs=3, space="PSUM") as ypool,
            tc.tile_pool(name="psum_s", bufs=1, space="PSUM") as spool,
            tc.tile_pool(name="elu", bufs=4) as elu_pool,
            tc.tile_pool(name="sout", bufs=1) as sout_pool,
        ):
            # Three DMA paths: SP + ACT (HWDGE) + GpSimd (SWDGE).  Consts are
            # chunked and interleaved into the tile stream just-in-time so
            # they never block item supply.
            dma_engines = [nc.sync, nc.scalar, nc.gpsimd]
            wtab_sb = const_pool.tile([128, 32 * 2 * 2 * 32], dt.bfloat16)
            mats_sb = const_pool.tile([128, n_seg * 2 * 2 * 128], dt.bfloat16)

            def load_mats(seg0, seg1, eng):
                sl = slice(seg0 * 512, seg1 * 512)
                eng.dma_start(mats_sb[:, sl], mats_d[:, sl])

            def load_wtab(c0, c1, eng):
                sl = slice(c0 * 128, c1 * 128)
                eng.dma_start(wtab_sb[:, sl], wtab_d[:, sl])

            # upfront: only what the first couple of tiles need
            load_mats(0, 2, nc.sync)
            load_wtab(0, 8, nc.scalar)
            load_mats(2, 4, nc.gpsimd)
            # interleaved const chunks: (emit_at_tile, fn)
            const_sched = {
                1: lambda eng: load_wtab(8, 20, eng),
                2: lambda eng: load_mats(4, 8, eng),
                3: lambda eng: load_wtab(20, 32, eng),
                5: lambda eng: load_mats(8, 12, eng),
                8: lambda eng: load_mats(12, 16, eng),
                11: lambda eng: load_mats(16, 20, eng),
                14: lambda eng: load_mats(20, 24, eng),
                17: lambda eng: load_mats(24, 28, eng),
                20: lambda eng: load_mats(28, 32, eng),
            }
            psum_s = spool.tile([128, TILE], dt.float32)

            def mat_sl(seg, kc, dc):
                off = ((seg * 2 + kc) * 2 + dc) * 128
                return mats_sb[:, off:off + 128]

            def w_sl(typ, kc, c):
                # c-major so tile t only depends on wtab chunk c = t % 32
                off = ((c * 2 + typ) * 2 + kc) * 32
                return wtab_sb[:, off:off + 32]

            for t in range(n_tiles):
                gl, r = divmod(t, tiles_per_seg)
                seg = 2 * gl + (0 if r == 0 else 1)
                typ = seg % 2
                grp, c = divmod(t, 32)

                if t in const_sched:
                    const_sched[t](dma_engines[(2 * t) % 3])

                xs = []
                for kc in range(2):
                    x = item_pool.tile([128, TILE], dt.bfloat16, tag="x")
                    eng = dma_engines[(2 * t + kc) % 3]
                    eng.dma_start(
                        x[:], items_d[kc * 128:(kc + 1) * 128,
                                      t * TILE:(t + 1) * TILE])
                    xs.append(x)

                # both d-chunks side by side in one 2-bank PSUM tile
                y = ypool.tile([128, 2 * TILE], dt.float32, tag="y")
                for dc in range(2):
                    ysl = y[:, dc * TILE:(dc + 1) * TILE]
                    nc.tensor.matmul(ysl, mat_sl(seg, 0, dc), xs[0][:],
                                     start=True, stop=False)
                    nc.tensor.matmul(ysl, mat_sl(seg, 1, dc), xs[1][:],
                                     start=False, stop=True)
                e_t = elu_pool.tile([128, 2 * TILE], dt.bfloat16, tag="e")
                nc.scalar.activation(e_t[:], y[:],
                                     mybir.ActivationFunctionType.Exp)
                el_t = elu_pool.tile([128, 2 * TILE], dt.bfloat16, tag="el")
                nc.vector._custom_dve(elup1, out=el_t[:], in0=e_t[:],
                                      in1=y[:], s0=1.0)

                out_sl = psum_s[32 * grp:32 * grp + 32, :]
                tp = (0, 32 * grp)
                nc.tensor.matmul(out_sl, w_sl(typ, 0, c),
                                 el_t[:, 0:TILE],
                                 start=(c == 0), stop=False,
                                 tile_position=tp, skip_group_check=True)
                nc.tensor.matmul(out_sl, w_sl(typ, 1, c),
                                 el_t[:, TILE:2 * TILE],
                                 start=False,
                                 stop=(c == 31 or t == n_tiles - 1),
                                 tile_position=tp, skip_group_check=True)

            s_sb = sout_pool.tile([128, TILE], dt.float32)
            nc.vector.tensor_copy(out=s_sb[:], in_=psum_s[:])
            nc.gpsimd.dma_start(s_d[:], s_sb[:])

    nc.compile()
    _BASS_CACHE[key] = nc
    return nc


# ----------------------------------------------------------------------------
# Host-side wrapper
# ----------------------------------------------------------------------------

def kernel(instruction_batch, distribution, node_prop_similarities,
           relation_similarity, node_attrs, edge_attrs,
           W_node_props, W_edge, w_node_score, w_rel_score,
           node_indices, edge_batch_indices, edge_indices):
    from concourse.bass_utils import run_bass_kernel_spmd

    ib = np.asarray(instruction_batch, dtype=np.float32)
    dist = np.asarray(distribution, dtype=np.float32)
    sim = np.asarray(node_prop_similarities, dtype=np.float32)
    rsim = np.asarray(relation_similarity, dtype=np.float32)
    na = np.asarray(node_attrs, dtype=np.float32)
    ea = np.asarray(edge_attrs, dtype=np.float32)
    Wp = np.asarray(W_node_props, dtype=np.float32)
    We = np.asarray(W_edge, dtype=np.float32)
    wn = np.asarray(w_node_score, dtype=np.float32)
    wr = np.asarray(w_rel_score, dtype=np.float32)
    ni = np.asarray(node_indices).astype(np.int64)
    ebi = np.asarray(edge_batch_indices).astype(np.int64)
    ei = np.asarray(edge_indices).astype(np.int64)
    src, dst = ei[0], ei[1]

    B = ib.shape[0]
    N = na.shape[0]
    G = B // N_CORES  # graphs per core

    cn = np.bincount(ni, minlength=B)
    ce = np.bincount(ebi, minlength=B)
    pad_n = max(TILE, int(-(-cn.max() // TILE)) * TILE)
    pad_e = max(TILE, int(-(-ce.max() // TILE)) * TILE)
    seg_items = pad_n + pad_e
    tiles_per_seg = seg_items // TILE
    n_tiles = G * tiles_per_seg
    m_pad = n_tiles * TILE
    assert n_tiles <= 128, "s accumulator bank overflow; shrink TILE padding"

    nstart = np.concatenate([[0], np.cumsum(cn)])
    eperm = np.argsort(ebi, kind="stable")
    estart = np.concatenate([[0], np.cumsum(ce)])

    # ---- item columns, transposed + bf16, padded per graph ----
    na_bf = na.astype(BF16)
    ea_bf = ea[eperm].astype(BF16)
    items = np.zeros((N_CORES, 2 * 128, m_pad), dtype=BF16)
    for g in range(B):
        dev, gl = divmod(g, G)
        c0 = gl * seg_items
        items[dev][:, c0:c0 + cn[g]] = na_bf[nstart[g]:nstart[g + 1]].T
        c1 = c0 + pad_n
        items[dev][:, c1:c1 + ce[g]] = ea_bf[estart[g]:estart[g + 1]].T

    # ---- per-graph matrices A[k, d] (instr folded in), bf16 ----
    C = np.einsum("gp,pde->gde", sim, Wp)
    A_node = (C * ib[:, :, None]).transpose(0, 2, 1)           # [g, k, d]
    A_edge = (We[None, :, :] * ib[:, :, None]).transpose(0, 2, 1)
    A = np.stack([A_node, A_edge], axis=1).astype(BF16)        # [g, 2, k, d]
    # blob[p, ((seg*2+kc)*2+dc)*128 + m] = A[seg][kc*128+p][dc*128+m]
    Ad = A.reshape(N_CORES, G * 2, 2, 128, 2, 128)             # dev,seg,kc,p,dc,m
    mats = np.ascontiguousarray(Ad.transpose(0, 3, 1, 2, 4, 5)
                                ).reshape(N_CORES, 128, -1)

    # ---- w tables: wtab[p, ((typ*2+kc)*32+c)*32+m] = w_typ[kc*128+p]*(m==c)
    wt = np.stack([wn, wr]).astype(np.float32)                  # [2, 256]
    eye = np.eye(32, dtype=np.float32)
    # wtab[k, ((c*2+typ)*2+kc)*32+m] = w_typ[kc*128+k] * (m == c)  (c-major)
    wtab = np.einsum("tk,cm->kctm", wt.reshape(2, 2, 128).reshape(4, 128), eye)
    wtab = np.ascontiguousarray(wtab.reshape(128, 32, 2, 2, 32)
                                ).reshape(128, 4 * 32 * 32).astype(BF16)

    # ---- run on 8 cores ----
    nc = _build_bass(n_tiles, tiles_per_seg, m_pad)
    in_maps = [{"items": items[d], "mats": mats[d], "wtab": wtab}
               for d in range(N_CORES)]
    res = run_bass_kernel_spmd(nc, in_maps, core_ids=list(range(N_CORES)))
    s_rows = np.stack([r["s_out"] for r in res.results])        # [8, 128, 512]

    # ---- unshard + finish on host ----
    sum_wn = float(wt[0].astype(BF16).astype(np.float32).sum())
    sum_wr = float(wt[1].astype(BF16).astype(np.float32).sum())
    state_logits = np.empty(N, np.float32)
    s_e = np.empty(ei.shape[1], np.float32)
    for g in range(B):
        dev, gl = divmod(g, G)
        rows = s_rows[dev].reshape(-1)[gl * seg_items:(gl + 1) * seg_items]
        state_logits[nstart[g]:nstart[g + 1]] = rows[:cn[g]] - sum_wn
        s_e[estart[g]:estart[g + 1]] = rows[pad_n:pad_n + ce[g]] - sum_wr

    rel_logits = np.bincount(dst[eperm], weights=dist[src[eperm]] * s_e,
                             minlength=N).astype(np.float32)

    def seg_softmax(x):
        mx = np.maximum.reduceat(x, nstart[:-1])
        ex = np.exp(x - mx[ni])
        sm = np.add.reduceat(ex, nstart[:-1])
        return ex / sm[ni]

    r = rsim[ni]
    out = r * seg_softmax(rel_logits) + (1.0 - r) * seg_softmax(state_logits)
    return out.astype(np.float32)


# revision 20
# speedup vs baseline: 1.5123x; 1.5123x over previous
"""Trainium2 Bass kernel for nn_NSMCell (GNN message passing).

Strategy
--------
The reference output is only [N]: a per-graph blend of two segment softmaxes
over per-node scalars.  Both scalars are of the form

    s_i = sum_d w_d * elu( M_g[d, :] @ x_i )

where for "node items" M_g = (sim[g] . W_node_props) * instr[g] and x = node
attr, and for "edge items" M_g = W_edge * instr[g] and x = edge attr.  The
per-graph matrices are built on the host (they are tiny); the device streams
all item columns through 4 matmuls + exp/min elu + a weighted partition
reduce.  Graphs (16 per core) are sharded across the 8 cores; every graph's
node and edge items are padded to fixed per-graph segment sizes so a single
NEFF serves all cores.  The edge-message scatter (index_add) collapses to a
host-side bincount of per-edge scalars, and the segment softmax + blend run
on the host over [N] values (negligible work).

Device layout per 512-item tile (d on partitions, 2 chunks of 128):
  y[d, e]   = A_seg[k, d]^T @ xT[k, e]      4 matmuls -> PSUM f32
  E         = exp(y)                        ScalarE, PSUM -> SBUF bf16
  R1        = max(y, 0) + 1                 VectorE tensor_scalar chain
  EL1       = min(E, R1) = elu(y) + 1       VectorE tensor_tensor
  s-rows   += (w (x) delta_c)^T @ EL1       2 matmuls into a PSUM s-bank
The s-bank accumulates one 512-wide row per tile (col-group trick), drained
once at the end; host subtracts sum(w) to undo the +1.
"""

import numpy as np
import ml_dtypes

BF16 = ml_dtypes.bfloat16
N_CORES = 8
D = 256
TILE = 512  # items per tile


# ----------------------------------------------------------------------------
# Bass kernel builder (one NEFF shared by all cores)
# ----------------------------------------------------------------------------

_BASS_CACHE = {}


def _get_elup1_op():
    """Register (once) a custom fused DVE op: out = min(in0, relu(in1) + s0).

    With in0 = exp(y) and in1 = y this computes elu(y) + 1 in a single
    VectorE pass, replacing a tensor_scalar + tensor_tensor pair."""
    from concourse import dve_ops
    from concourse.dve_spec import (Spec, Src0, Src1, C0, relu, minn, lower,
                                    _has_src1)
    from concourse.dve_uop import DveOpSpec

    for o in dve_ops.OPS:
        if o.name == "ELUP1_ANT":
            return o

    def ref(in0, in1, s0, s1, imm2):
        return np.minimum(
            in0.astype(np.float32),
            np.maximum(in1.astype(np.float32), 0.0) + s0,
        ).astype(np.float32)

    spec = Spec(body=minn(Src0, relu(Src1) + C0), reference=ref)
    row = dve_ops._CUSTOM_DVE_ROW_BASE + len(dve_ops.OPS)
    shas = {}
    for ver in ("v3", "v4"):
        uops = lower(spec, ver=ver)
        shas[ver] = DveOpSpec(name="ELUP1_ANT", opcode=row, uops=uops,
                              rd1_en=_has_src1(spec)).sha(ver)
    op = dve_ops.DveOp("ELUP1_ANT", spec, subdim=False, uops_sha=shas)
    dve_ops.OPS.append(op)
    dve_ops.CUSTOM_DVE_SPECS[op.name] = op.spec
    dve_ops._SUB_OPCODE_FOR_NAME[op.name] = row
    return op


def _build_bass(n_tiles, tiles_per_seg, m_pad):
    """Build the Tile/Bass program.

    n_tiles: number of 512-item tiles per core (== 16 graphs * tiles_per_seg)
    tiles_per_seg: tiles per graph segment-pair (1 node tile + edge tiles)
    """
    key = (n_tiles, tiles_per_seg, m_pad)
    if key in _BASS_CACHE:
        return _BASS_CACHE[key]

    import concourse.mybir as mybir
    import concourse.tile as tile
    from concourse import bacc

    dt = mybir.dt
    n_seg = 32  # 16 graphs x (node, edge)
    assert n_tiles == 16 * tiles_per_seg
    assert m_pad == n_tiles * TILE
    assert n_tiles <= 128

    elup1 = _get_elup1_op()
    nc = bacc.Bacc("TRN2", target_bir_lowering=False)
    items_d = nc.dram_tensor("items", [2 * 128, m_pad], dt.bfloat16,
                             kind="ExternalInput")
    mats_d = nc.dram_tensor("mats", [128, n_seg * 2 * 2 * 128], dt.bfloat16,
                            kind="ExternalInput")
    wtab_d = nc.dram_tensor("wtab", [128, 2 * 2 * 32 * 32], dt.bfloat16,
                            kind="ExternalInput")
    s_d = nc.dram_tensor("s_out", [128, TILE], dt.float32,
                         kind="ExternalOutput")

    with tile.TileContext(nc) as tc:
        with (
            tc.tile_pool(name="const", bufs=1) as const_pool,
            tc.tile_pool(name="items", bufs=10) as item_pool,
            tc.tile_pool(name="psum_y", bufs=3, space="PSUM") as ypool,
            tc.tile_pool(name="psum_s", bufs=1, space="PSUM") as spool,
            tc.tile_pool(name="elu", bufs=4) as elu_pool,
            tc.tile_pool(name="sout", bufs=1) as sout_pool,
        ):
            # Items: kc0 on the SP HWDGE path, kc1 on the GpSimd SWDGE path.
            # Consts: all on the ACT HWDGE path, chunked in just-in-time
            # order so early tiles' deps land first.
            wtab_sb = const_pool.tile([128, 32 * 2 * 2 * 32], dt.bfloat16)
            mats_sb = const_pool.tile([128, n_seg * 2 * 2 * 128], dt.bfloat16)

            def load_mats(seg0, seg1):
                sl = slice(seg0 * 512, seg1 * 512)
                nc.scalar.dma_start(mats_sb[:, sl], mats_d[:, sl])

            def load_wtab(c0, c1):
                sl = slice(c0 * 128, c1 * 128)
                nc.scalar.dma_start(wtab_sb[:, sl], wtab_d[:, sl])

            load_mats(0, 2)
            load_wtab(0, 8)
            load_mats(2, 6)
            load_wtab(8, 32)
            load_mats(6, 12)
            load_mats(12, 20)
            load_mats(20, 32)
            psum_s = spool.tile([128, TILE], dt.float32)

            def mat_sl(seg, kc, dc):
                off = ((seg * 2 + kc) * 2 + dc) * 128
                return mats_sb[:, off:off + 128]

            def w_sl(typ, kc, c):
                # c-major so tile t only depends on wtab chunk c = t % 32
                off = ((c * 2 + typ) * 2 + kc) * 32
                return wtab_sb[:, off:off + 32]

            for t in range(n_tiles):
                gl, r = divmod(t, tiles_per_seg)
                seg = 2 * gl + (0 if r == 0 else 1)
                typ = seg % 2
                grp, c = divmod(t, 32)

                xs = []
                for kc in range(2):
                    x = item_pool.tile([128, TILE], dt.bfloat16, tag="x")
                    eng = nc.sync if kc == 0 else nc.gpsimd
                    eng.dma_start(
                        x[:], items_d[kc * 128:(kc + 1) * 128,
                                      t * TILE:(t + 1) * TILE])
                    xs.append(x)

                # both d-chunks side by side in one 2-bank PSUM tile
                y = ypool.tile([128, 2 * TILE], dt.float32, tag="y")
                for dc in range(2):
                    ysl = y[:, dc * TILE:(dc + 1) * TILE]
                    nc.tensor.matmul(ysl, mat_sl(seg, 0, dc), xs[0][:],
                                     start=True, stop=False)
                    nc.tensor.matmul(ysl, mat_sl(seg, 1, dc), xs[1][:],
                                     start=False, stop=True)
                e_t = elu_pool.tile([128, 2 * TILE], dt.bfloat16, tag="e")
                nc.scalar.activation(e_t[:], y[:],
                                     mybir.ActivationFunctionType.Exp)
                el_t = elu_pool.tile([128, 2 * TILE], dt.bfloat16, tag="el")
                nc.vector._custom_dve(elup1, out=el_t[:], in0=e_t[:],
                                      in1=y[:], s0=1.0)

                out_sl = psum_s[32 * grp:32 * grp + 32, :]
                tp = (0, 32 * grp)
                nc.tensor.matmul(out_sl, w_sl(typ, 0, c),
                                 el_t[:, 0:TILE],
                                 start=(c == 0), stop=False,
                                 tile_position=tp, skip_group_check=True)
                nc.tensor.matmul(out_sl, w_sl(typ, 1, c),
                                 el_t[:, TILE:2 * TILE],
                                 start=False,
                                 stop=(c == 31 or t == n_tiles - 1),
                                 tile_position=tp, skip_group_check=True)

            s_sb = sout_pool.tile([128, TILE], dt.float32)
            nc.vector.tensor_copy(out=s_sb[:], in_=psum_s[:])
            nc.gpsimd.dma_start(s_d[:], s_sb[:])

    nc.compile()
    _BASS_CACHE[key] = nc
    return nc


# ----------------------------------------------------------------------------
# Host-side wrapper
# ----------------------------------------------------------------------------

def kernel(instruction_batch, distribution, node_prop_similarities,
           relation_similarity, node_attrs, edge_attrs,
           W_node_props, W_edge, w_node_score, w_rel_score,
           node_indices, edge_batch_indices, edge_indices):
    from concourse.bass_utils import run_bass_kernel_spmd

    ib = np.asarray(instruction_batch, dtype=np.float32)
    dist = np.asarray(distribution, dtype=np.float32)
    sim = np.asarray(node_prop_similarities, dtype=np.float32)
    rsim = np.asarray(relation_similarity, dtype=np.float32)
    na = np.asarray(node_attrs, dtype=np.float32)
    ea = np.asarray(edge_attrs, dtype=np.float32)
    Wp = np.asarray(W_node_props, dtype=np.float32)
    We = np.asarray(W_edge, dtype=np.float32)
    wn = np.asarray(w_node_score, dtype=np.float32)
    wr = np.asarray(w_rel_score, dtype=np.float32)
    ni = np.asarray(node_indices).astype(np.int64)
    ebi = np.asarray(edge_batch_indices).astype(np.int64)
    ei = np.asarray(edge_indices).astype(np.int64)
    src, dst = ei[0], ei[1]

    B = ib.shape[0]
    N = na.shape[0]
    G = B // N_CORES  # graphs per core

    cn = np.bincount(ni, minlength=B)
    ce = np.bincount(ebi, minlength=B)
    pad_n = max(TILE, int(-(-cn.max() // TILE)) * TILE)
    pad_e = max(TILE, int(-(-ce.max() // TILE)) * TILE)
    seg_items = pad_n + pad_e
    tiles_per_seg = seg_items // TILE
    n_tiles = G * tiles_per_seg
    m_pad = n_tiles * TILE
    assert n_tiles <= 128, "s accumulator bank overflow; shrink TILE padding"

    nstart = np.concatenate([[0], np.cumsum(cn)])
    eperm = np.argsort(ebi, kind="stable")
    estart = np.concatenate([[0], np.cumsum(ce)])

    # ---- item columns, transposed + bf16, padded per graph ----
    na_bf = na.astype(BF16)
    ea_bf = ea[eperm].astype(BF16)
    items = np.zeros((N_CORES, 2 * 128, m_pad), dtype=BF16)
    for g in range(B):
        dev, gl = divmod(g, G)
        c0 = gl * seg_items
        items[dev][:, c0:c0 + cn[g]] = na_bf[nstart[g]:nstart[g + 1]].T
        c1 = c0 + pad_n
        items[dev][:, c1:c1 + ce[g]] = ea_bf[estart[g]:estart[g + 1]].T

    # ---- per-graph matrices A[k, d] (instr folded in), bf16 ----
    C = np.einsum("gp,pde->gde", sim, Wp)
    A_node = (C * ib[:, :, None]).transpose(0, 2, 1)           # [g, k, d]
    A_edge = (We[None, :, :] * ib[:, :, None]).transpose(0, 2, 1)
    A = np.stack([A_node, A_edge], axis=1).astype(BF16)        # [g, 2, k, d]
    # blob[p, ((seg*2+kc)*2+dc)*128 + m] = A[seg][kc*128+p][dc*128+m]
    Ad = A.reshape(N_CORES, G * 2, 2, 128, 2, 128)             # dev,seg,kc,p,dc,m
    mats = np.ascontiguousarray(Ad.transpose(0, 3, 1, 2, 4, 5)
                                ).reshape(N_CORES, 128, -1)

    # ---- w tables: wtab[p, ((typ*2+kc)*32+c)*32+m] = w_typ[kc*128+p]*(m==c)
    wt = np.stack([wn, wr]).astype(np.float32)                  # [2, 256]
    eye = np.eye(32, dtype=np.float32)
    # wtab[k, ((c*2+typ)*2+kc)*32+m] = w_typ[kc*128+k] * (m == c)  (c-major)
    wtab = np.einsum("tk,cm->kctm", wt.reshape(2, 2, 128).reshape(4, 128), eye)
    wtab = np.ascontiguousarray(wtab.reshape(128, 32, 2, 2, 32)
                                ).reshape(128, 4 * 32 * 32).astype(BF16)

    # ---- run on 8 cores ----
    nc = _build_bass(n_tiles, tiles_per_seg, m_pad)
    in_maps = [{"items": items[d], "mats": mats[d], "wtab": wtab}
               for d in range(N_CORES)]
    res = run_bass_kernel_spmd(nc, in_maps, core_ids=list(range(N_CORES)))
    s_rows = np.stack([r["s_out"] for r in res.results])        # [8, 128, 512]

    # ---- unshard + finish on host ----
    sum_wn = float(wt[0].astype(BF16).astype(np.float32).sum())
    sum_wr = float(wt[1].astype(BF16).astype(np.float32).sum())
    state_logits = np.empty(N, np.float32)
    s_e = np.empty(ei.shape[1], np.float32)
    for g in range(B):
        dev, gl = divmod(g, G)
        rows = s_rows[dev].reshape(-1)[gl * seg_items:(gl + 1) * seg_items]
        state_logits[nstart[g]:nstart[g + 1]] = rows[:cn[g]] - sum_wn
        s_e[estart[g]:estart[g + 1]] = rows[pad_n:pad_n + ce[g]] - sum_wr

    rel_logits = np.bincount(dst[eperm], weights=dist[src[eperm]] * s_e,
                             minlength=N).astype(np.float32)

    def seg_softmax(x):
        mx = np.maximum.reduceat(x, nstart[:-1])
        ex = np.exp(x - mx[ni])
        sm = np.add.reduceat(ex, nstart[:-1])
        return ex / sm[ni]

    r = rsim[ni]
    out = r * seg_softmax(rel_logits) + (1.0 - r) * seg_softmax(state_logits)
    return out.astype(np.float32)
